# revision 20
# baseline (speedup 1.0000x reference)
"""GCN + DiffPool kernel for Trainium2, data-parallel over graphs across 8 NeuronCores.

Model (per graph, n=150 nodes):
  Z1 = relu(An @ (x @ W1) + b1)          An = D^-1/2 (A+I) D^-1/2
  Z2 = relu(An @ (Z1 @ W2) + b2)
  S  = softmax(An @ (Z2 @ Wa) + ba)      [n, 25]
  Zp = S^T @ Z2 ; Ap = S^T @ (A @ S)
  H  = relu(Anp @ (Zp @ Wp) + bp)        pooled GCN, 25 cluster-nodes
  logits = (sum_rows H) @ Wc + bc

Sharding: 64 graphs -> 8 devices x 8 graphs; each device gets its graphs'
150x150 diagonal blocks of A+I (node chunks c0=[0:128], c1=[128:150] on
partitions) and node rows of x (feature-major). Final [8,10] logits per
device concatenate on host.

All-node-major dataflow. Every activation keeps nodes on partitions, so the
row normalization factor d = rsqrt(deg+1) is a per-partition scale:
    Z1d = d_j * Z1[j,:] = relu(d_j^2 * psum)         (d>0 commutes with relu)
with psum = sum_i Ah[i,j] (d_i M1[i,h]) + dinv_j*b1[h]; the bias rides an
augmented contraction row (Ah row 22 = dinv_j, M1d row 22 = b1), and the
column factor d_j^2 = 1/(deg_j+1) is applied at the PSUM drain. The
propagate matmuls use lhsT = Ah[i, j-slice] directly (A+I is symmetric), so
no An matrix and no transposes are ever materialized. AS = A@S is recovered
from (A+I)@S - S. The pooled stage folds dp past relu into the readout:
  G @ Wc = sum_c' dp_c' relu(psum_h)[:,c'] @ Wc  ->  per-cluster matmul
then a ones-contraction matmul collapses clusters, with bc on an aug row.
"""

import numpy as np

import concourse.bass as bass
import concourse.mybir as mybir
import concourse.tile as tile
from concourse import bacc
from concourse.bass_utils import run_bass_kernel_spmd

F32 = mybir.dt.float32
BF16 = mybir.dt.bfloat16
U32 = mybir.dt.uint32
AF = mybir.ActivationFunctionType
AL = mybir.AluOpType

MMDT = BF16

N_NODES = 9600
N_FEAT = 128
HIDDEN = 64
CLUSTERS = 25
NUM_CLASSES = 10
B_GRAPHS = 64
NPG = 150            # nodes per graph
DEV = 8              # devices
GPD = 8              # graphs per device
C0, C1 = 128, 22     # node partition chunks

# wpk (bf16) column offsets
WP_W1 = 0                       # [128, 64]
WP_W2A = WP_W1 + HIDDEN         # [65, 64]  row 64 = b2; row 65 = b1; row 66 = bp
WP_WAA = WP_W2A + HIDDEN        # [65, 25]  row 64 = ba
WP_WP = WP_WAA + CLUSTERS       # [64, 64]
WP_WC = WP_WP + HIDDEN          # [64, 10]
WP_COLS = WP_WC + NUM_CLASSES

# fpk (fp32) column offsets
FP_ONES = 0                     # rows 0:26 = 1.0 (ones contraction col)
FP_BC = 1                       # row 0, cols 1:81 = tile(bc, 8)
FP_COLS = FP_BC + GPD * NUM_CLASSES

_CACHE = {}


def _chunk(c):
    return (0, C0) if c == 0 else (C0, C1)


def build_nc():
    nc = bacc.Bacc("TRN2", target_bir_lowering=False, debug=False, num_devices=DEV)

    def din(name, shape, dt=MMDT):
        return nc.dram_tensor(name, shape, dt, kind="ExternalInput").ap()

    xTa = din("xTa", [N_FEAT, GPD // 2, NPG])
    xTb = din("xTb", [N_FEAT, GPD // 2, NPG])
    a0a = din("a0a", [C0, GPD // 2, NPG])   # rows 0:128 of A+I, graphs 0:4
    a0b = din("a0b", [C0, GPD // 2, NPG])   # rows 0:128 of A+I, graphs 4:8
    a1 = din("a1", [C1, GPD, NPG])          # rows 128:150 of A+I blocks
    wpk = din("wpk", [N_FEAT, WP_COLS])
    fpk = din("fpk", [N_FEAT, FP_COLS], F32)
    outd = nc.dram_tensor("out", [GPD * NUM_CLASSES], F32, kind="ExternalOutput").ap()

    with tile.TileContext(nc) as tc:
        with (
            tc.tile_pool(name="cst", bufs=1) as cst,
            tc.tile_pool(name="act", bufs=1) as act,
            tc.tile_pool(name="ps", bufs=3, space="PSUM") as ps,
            tc.tile_pool(name="psu", bufs=2, space="PSUM") as psu,
            tc.tile_pool(name="pst", bufs=1, space="PSUM") as pst,
            tc.tile_pool(name="pw", bufs=1, space="PSUM") as pwp,
            tc.tile_pool(name="dram", bufs=1, space="DRAM") as dram,
        ):
            H2 = GPD // 2

            # ---- input DMAs. Only the gpsimd SWDGE queue moves big multi-row
            # transfers at full rate; the adjacency (heads the degree->d->aug
            # critical chain) goes there first. sync HWDGE takes the rest.
            s_a0 = cst.tile([C0, GPD, NPG], MMDT, tag="a0")
            nc.gpsimd.dma_start(out=s_a0[:, 0:H2, :], in_=a0a)
            nc.gpsimd.dma_start(out=s_a0[:, H2:GPD, :], in_=a0b)
            s_a1 = cst.tile([C1 + 1, GPD, NPG], MMDT, tag="a1")
            nc.gpsimd.dma_start(out=s_a1[0:C1, :, :], in_=a1)
            s_wpk = cst.tile([N_FEAT, WP_COLS], MMDT, tag="wpk")
            nc.sync.dma_start(out=s_wpk[:], in_=wpk)
            s_xT = cst.tile([N_FEAT, GPD, NPG], MMDT, tag="xT")
            nc.sync.dma_start(out=s_xT[:, 0:H2, :], in_=xTa)
            nc.gpsimd.dma_start(out=s_xT[:, H2:GPD, :], in_=xTb)
            s_fpk = cst.tile([N_FEAT, FP_COLS], F32, tag="fpk")
            nc.sync.dma_start(out=s_fpk[:], in_=fpk)

            # ---- PE warmup: keep HAM busy while DMAs land ------------------
            warm = cst.tile([C0, 256], MMDT, tag="warm")
            nc.gpsimd.memset(warm[:], 1.0)
            pwt = pwp.tile([C0, 256], F32, tag="pw")
            for _ in range(12):
                nc.tensor.matmul(pwt[:], warm[:, 0:C0], warm[:],
                                 start=True, stop=True)

            # identity built on device (saves a 32KB const load)
            idt = cst.tile([C0, C0], MMDT, tag="idt")
            nc.gpsimd.affine_select(idt[:], warm[:, 0:C0], [[1, C0]],
                                    AL.is_equal, 0.0, base=0,
                                    channel_multiplier=-1)

            s_a = (s_a0, s_a1)
            s_W1 = s_wpk[:, WP_W1:WP_W1 + HIDDEN]
            s_W2a = s_wpk[0:HIDDEN + 1, WP_W2A:WP_W2A + HIDDEN]
            s_Waa = s_wpk[0:HIDDEN + 1, WP_WAA:WP_WAA + CLUSTERS]
            s_Wp = s_wpk[0:HIDDEN, WP_WP:WP_WP + HIDDEN]
            s_Wc = s_wpk[0:HIDDEN, WP_WC:WP_WC + NUM_CLASSES]
            s_ones26 = s_fpk[0:CLUSTERS + 1, FP_ONES:FP_ONES + 1]

            # ---- degrees: rowsum(A+I) = deg+1 on partitions ----------------
            degc = act.tile([C0, 2 * GPD], F32, tag="degc")
            nc.vector.memset(degc[0:C0, GPD:2 * GPD], 1.0)
            nc.vector.reduce_sum(out=degc[:, 0:H2], in_=s_a0[:, 0:H2, :],
                                 axis=mybir.AxisListType.X)
            nc.vector.reduce_sum(out=degc[:, H2:GPD], in_=s_a0[:, H2:GPD, :],
                                 axis=mybir.AxisListType.X)
            nc.vector.reduce_sum(out=degc[0:C1, GPD:2 * GPD], in_=s_a1[0:C1, :, :],
                                 axis=mybir.AxisListType.X)

            def emit_rsqrt(x, rows, cols):
                """rsqrt via exp(-0.5*ln(x)) on ACT -- ln/exp share one
                table set with the softmax Exp, so no table thrash."""
                lg = act.tile([rows, cols], F32, tag=f"lg_{id(x)}")
                nc.scalar.activation(lg[:], x[:], AF.Ln)
                s = act.tile([rows, cols], F32, tag=f"rs_{id(x)}")
                nc.scalar.activation(s[:], lg[:], AF.Exp, scale=-0.5)
                return s

            dcomb = emit_rsqrt(degc, C0, 2 * GPD)          # d = rsqrt(deg+1)
            d2comb = act.tile([C0, 2 * GPD], F32, tag="d2c")
            nc.vector.reciprocal(d2comb[:], degc[:])       # d^2 = 1/(deg+1)
            dinvc = act.tile([C0, 2 * GPD], F32, tag="dic")
            nc.vector.tensor_mul(dinvc[:], dcomb[:], degc[:])   # 1/d = sqrt(deg+1)
            dinvb = act.tile([C0, 2 * GPD], MMDT, tag="dib")
            nc.vector.tensor_copy(dinvb[:], dinvc[:])

            s_d = [dcomb[:, 0:GPD], dcomb[0:C1, GPD:2 * GPD]]
            s_d2 = [d2comb[:, 0:GPD], d2comb[0:C1, GPD:2 * GPD]]
            s_dinvb = [dinvb[:, 0:GPD], dinvb[0:C1, GPD:2 * GPD]]

            # ---- dinv as a free-dim row via PE transpose + DRAM hop --------
            p_dt = pst.tile([GPD * 2, 160], MMDT, tag="ptr")
            nc.tensor.transpose(p_dt[0:GPD, 0:C0], s_dinvb[0][:], idt[:])
            nc.tensor.transpose(p_dt[0:GPD, C0:NPG], s_dinvb[1][:],
                                idt[0:C1, 0:C1])
            dTrow = act.tile([GPD, NPG], MMDT, tag="dTrow")
            nc.vector.tensor_copy(dTrow[:], p_dt[0:GPD, 0:NPG])
            dTd = dram.tile([GPD * NPG], MMDT, tag="dTd")
            nc.sync.dma_start(out=dTd[:].rearrange("(g j) -> g j", g=GPD),
                              in_=dTrow[:])
            dinv_row = dTd[:].rearrange("(g j) -> g j", g=GPD)[None, :, :]
            # aug row 22 of the chunk-1 adjacency: dinv_j
            nc.sync.dma_start(out=s_a1[C1:C1 + 1, :, :], in_=dinv_row)

            # ---- M1 = (X @ W1) * d_row, node-major; aug row = b1 -----------
            m1 = []
            for c, cn in ((0, C0), (1, C1)):
                off, _ = _chunk(c)
                p = ps.tile([C0, GPD, HIDDEN], F32, tag="ps")
                for g in range(GPD):
                    nc.tensor.matmul(p[0:cn, g, :],
                                     s_xT[:, g, off:off + cn], s_W1,
                                     start=True, stop=True)
                rows = cn + (1 if c == 1 else 0)
                o = act.tile([rows, GPD, HIDDEN], MMDT, tag=f"m1_{c}")
                dbc = s_d[c][:][:, :, None].broadcast_to((cn, GPD, HIDDEN))
                nc.vector.tensor_mul(o[0:cn, :, :], p[0:cn, :, :], dbc)
                if c == 1:
                    b1b = wpk[HIDDEN + 1:HIDDEN + 2, WP_W2A:WP_W2A + HIDDEN] \
                        [:, None, :].broadcast_to((1, GPD, HIDDEN))
                    nc.gpsimd.dma_start(out=o[C1:C1 + 1, :, :], in_=b1b)
                m1.append(o)

            def prop_nm(rhs_tiles, d2s, name, fout=HIDDEN):
                """Z[j,h] = relu(d_j^2 * sum_i Ah_aug[i,j] rhs_aug[i,h]).
                rhs tiles: (c0 [128,g,fout], c1 [23,g,fout] w/ aug row)."""
                outs = []
                for jc, jn in ((0, C0), (1, C1)):
                    joff, _ = _chunk(jc)
                    p = ps.tile([C0, GPD, fout], F32, tag="ps")
                    for g in range(GPD):
                        nc.tensor.matmul(p[0:jn, g, :],
                                         s_a0[:, g, joff:joff + jn],
                                         rhs_tiles[0][0:C0, g, :],
                                         start=True, stop=False)
                        nc.tensor.matmul(p[0:jn, g, :],
                                         s_a1[0:C1 + 1, g, joff:joff + jn],
                                         rhs_tiles[1][0:C1 + 1, g, :],
                                         start=False, stop=True)
                    o = act.tile([jn, GPD, fout], MMDT, tag=f"{name}{jc}")
                    d2bc = d2s[jc][:][:, :, None].broadcast_to((jn, GPD, fout))
                    nc.vector.scalar_tensor_tensor(
                        o[:], p[0:jn, :, :], 0.0, d2bc, AL.max, AL.mult)
                    outs.append(o)
                return outs

            # ---- layer 1: Z1d = d * relu(An@M1 + b1) -----------------------
            z1d = prop_nm(m1, s_d2, "z1d")

            # ---- U = raw(An @ Z1), feature-major; aug row 64 = dinv --------
            def an_prop_fm(lhs_tiles, name):
                o = act.tile([HIDDEN + 1, GPD, NPG], MMDT, tag=name)
                nc.scalar.dma_start(out=o[HIDDEN:HIDDEN + 1, :, :], in_=dinv_row)
                for h in range(0, GPD, 2):
                    p = psu.tile([HIDDEN, 2, 256], F32, tag="psu")
                    for gg in range(2):
                        g = h + gg
                        for c, cn in ((0, C0), (1, C1)):
                            off, _ = _chunk(c)
                            nc.tensor.matmul(p[:, gg, 0:NPG],
                                             lhs_tiles[c][0:cn, g, :],
                                             s_a[c][0:cn, g, :],
                                             start=(c == 0), stop=(c == 1))
                    nc.scalar.copy(o[0:HIDDEN, h:h + 2, :], p[:, :, 0:NPG])
                return o

            u = an_prop_fm(z1d, "u")

            # ---- layer 2: Z2d = d * relu((U@W2)*d + b2) --------------------
            def w_stage_nm(lhs_fm, w_aug, d2s, name, fout=HIDDEN, relu=True):
                """out[j,:] = drain(d_j^2 * sum_h lhs_aug[h,j] w_aug[h,:])."""
                outs = []
                for jc, jn in ((0, C0), (1, C1)):
                    joff, _ = _chunk(jc)
                    p = ps.tile([C0, GPD, fout], F32, tag="ps")
                    for g in range(GPD):
                        nc.tensor.matmul(p[0:jn, g, :],
                                         lhs_fm[0:HIDDEN + 1, g, joff:joff + jn],
                                         w_aug, start=True, stop=True)
                    outs.append(p)
                return outs

            p2 = w_stage_nm(u, s_W2a, s_d2, "p2")
            z2d = []
            for jc, jn in ((0, C0), (1, C1)):
                o = act.tile([jn, GPD, HIDDEN], MMDT, tag=f"z2d{jc}")
                d2bc = s_d2[jc][:][:, :, None].broadcast_to((jn, GPD, HIDDEN))
                nc.vector.scalar_tensor_tensor(
                    o[:], p2[jc][0:jn, :, :], 0.0, d2bc, AL.max, AL.mult)
                z2d.append(o)

            # ---- T = raw(An @ Z2), then P = T@Wa, softmax ------------------
            t = an_prop_fm(z2d, "t")
            pp = w_stage_nm(t, s_Waa, None, "pp", fout=CLUSTERS)

            s_S, s_Si = [], []
            for jc, jn in ((0, C0), (1, C1)):
                pm = act.tile([jn, GPD, CLUSTERS], F32, tag=f"pm{jc}")
                dbc = s_d[jc][:][:, :, None].broadcast_to((jn, GPD, CLUSTERS))
                nc.vector.tensor_mul(pm[:], pp[jc][0:jn, :, :], dbc)
                e = act.tile([jn, GPD, CLUSTERS], F32, tag=f"e{jc}")
                nc.scalar.activation(e[:], pm[:], AF.Exp)
                ssum = act.tile([jn, GPD], F32, tag=f"ssum{jc}")
                nc.vector.reduce_sum(out=ssum[:], in_=e[:],
                                     axis=mybir.AxisListType.X)
                rs = act.tile([jn, GPD], F32, tag=f"rsx{jc}")
                nc.vector.reciprocal(rs[:], ssum[:])
                s = act.tile([jn, GPD, CLUSTERS], MMDT, tag=f"s{jc}")
                nc.vector.tensor_mul(
                    s[:], e[:], rs[:][:, :, None].broadcast_to((jn, GPD, CLUSTERS)))
                s_S.append(s)
                si = act.tile([jn, GPD, CLUSTERS], MMDT, tag=f"si{jc}")
                dib = s_dinvb[jc][:][:, :, None].broadcast_to((jn, GPD, CLUSTERS))
                nc.vector.tensor_mul(si[:], s[:], dib)
                s_Si.append(si)

            # ---- AS = (A+I)@S - S, node-major ------------------------------
            s_AS = []
            for jc, jn in ((0, C0), (1, C1)):
                joff, _ = _chunk(jc)
                p = ps.tile([C0, GPD, CLUSTERS], F32, tag="ps")
                for g in range(GPD):
                    for c, cn in ((0, C0), (1, C1)):
                        nc.tensor.matmul(p[0:jn, g, :],
                                         s_a[c][0:cn, g, joff:joff + jn],
                                         s_S[c][0:cn, g, :],
                                         start=(c == 0), stop=(c == 1))
                o = act.tile([jn, GPD, CLUSTERS], MMDT, tag=f"as{jc}")
                nc.vector.tensor_sub(o[:], p[0:jn, :, :], s_S[jc][:])
                s_AS.append(o)

            # ---- Ap = S^T @ AS ; Zp^T = Z2^T @ S ---------------------------
            p_ap = ps.tile([CLUSTERS, GPD, CLUSTERS], F32, tag="ps")
            for g in range(GPD):
                for c, cn in ((0, C0), (1, C1)):
                    nc.tensor.matmul(p_ap[:, g, :], s_S[c][0:cn, g, :],
                                     s_AS[c][0:cn, g, :],
                                     start=(c == 0), stop=(c == 1))
            p_zp = ps.tile([HIDDEN, GPD, CLUSTERS], F32, tag="ps")
            for g in range(GPD):
                for c, cn in ((0, C0), (1, C1)):
                    nc.tensor.matmul(p_zp[:, g, :], z2d[c][0:cn, g, :],
                                     s_Si[c][0:cn, g, :],
                                     start=(c == 0), stop=(c == 1))
            s_Zp = act.tile([HIDDEN, GPD, CLUSTERS], MMDT, tag="zp")
            nc.scalar.copy(s_Zp[:], p_zp[:])

            # ---- pooled normalization --------------------------------------
            degp = act.tile([CLUSTERS, GPD], F32, tag="degp")
            nc.vector.reduce_sum(out=degp[:], in_=p_ap[:],
                                 axis=mybir.AxisListType.X)
            nc.vector.tensor_scalar_add(degp[:], degp[:], 1.0)
            dp = emit_rsqrt(degp, CLUSTERS, GPD)
            dinvp = act.tile([CLUSTERS, GPD], MMDT, tag="dinvp")
            dinvpf = act.tile([CLUSTERS, GPD], F32, tag="dinvpf")
            nc.vector.tensor_mul(dinvpf[:], dp[:], degp[:])
            nc.vector.tensor_copy(dinvp[:], dinvpf[:])

            p_dp = pst.tile([GPD * 2, 160], MMDT, tag="ptr")
            nc.tensor.transpose(p_dp[0:GPD, 0:CLUSTERS], dinvp[:],
                                idt[0:CLUSTERS, 0:CLUSTERS])
            dprow = act.tile([GPD, CLUSTERS], MMDT, tag="dprow")
            nc.vector.tensor_copy(dprow[:], p_dp[0:GPD, 0:CLUSTERS])
            dpd = dram.tile([GPD * CLUSTERS], MMDT, tag="dpd")
            nc.sync.dma_start(out=dpd[:].rearrange("(g c) -> g c", g=GPD),
                              in_=dprow[:])

            # Ahp = Ap + I, with aug row 25 = dinvp_col
            ahp = act.tile([CLUSTERS + 1, GPD, CLUSTERS], MMDT, tag="ahp")
            nc.sync.dma_start(
                out=ahp[CLUSTERS:CLUSTERS + 1, :, :],
                in_=dpd[:].rearrange("(g c) -> g c", g=GPD)[None, :, :])
            id25b = idt[0:CLUSTERS, 0:CLUSTERS][:, None, :] \
                .broadcast_to((CLUSTERS, GPD, CLUSTERS))
            nc.vector.tensor_add(ahp[0:CLUSTERS, :, :], p_ap[:], id25b)

            # ---- pooled GCN ------------------------------------------------
            p_zw = ps.tile([CLUSTERS, GPD, HIDDEN], F32, tag="ps")
            for g in range(GPD):
                nc.tensor.matmul(p_zw[:, g, :], s_Zp[:, g, :], s_Wp,
                                 start=True, stop=True)
            zwd = act.tile([CLUSTERS + 1, GPD, HIDDEN], MMDT, tag="zwd")
            bpb = wpk[HIDDEN + 2:HIDDEN + 3, WP_W2A:WP_W2A + HIDDEN] \
                [:, None, :].broadcast_to((1, GPD, HIDDEN))
            nc.gpsimd.dma_start(out=zwd[CLUSTERS:CLUSTERS + 1, :, :], in_=bpb)
            dpbc = dp[:][:, :, None].broadcast_to((CLUSTERS, GPD, HIDDEN))
            nc.vector.tensor_mul(zwd[0:CLUSTERS, :, :], p_zw[:], dpbc)

            p_h = ps.tile([HIDDEN, GPD, CLUSTERS], F32, tag="ps")
            for g in range(GPD):
                nc.tensor.matmul(p_h[:, g, :], zwd[0:CLUSTERS + 1, g, :],
                                 ahp[0:CLUSTERS + 1, g, :],
                                 start=True, stop=True)
            y = act.tile([HIDDEN, GPD, CLUSTERS], MMDT, tag="y")
            nc.scalar.activation(y[:], p_h[:], AF.Relu)

            # ---- readout: logits = sum_c' dp_c' (Y^T Wc)[c',:] + bc --------
            p_l = ps.tile([CLUSTERS, GPD, NUM_CLASSES], F32, tag="ps")
            for g in range(GPD):
                nc.tensor.matmul(p_l[:, g, :], y[:, g, :], s_Wc,
                                 start=True, stop=True)
            ldp = act.tile([CLUSTERS + 1, GPD, NUM_CLASSES], F32, tag="ldp")
            nc.sync.dma_start(
                out=ldp[CLUSTERS:CLUSTERS + 1, :, :],
                in_=fpk[0:1, FP_BC:FP_BC + GPD * NUM_CLASSES]
                .rearrange("one (g c) -> one g c", g=GPD))
            dpb2 = dp[:][:, :, None].broadcast_to((CLUSTERS, GPD, NUM_CLASSES))
            nc.vector.tensor_mul(ldp[0:CLUSTERS, :, :], p_l[:], dpb2)

            p_f = pst.tile([GPD * NUM_CLASSES, 1], F32, tag="pf")
            nc.tensor.matmul(
                p_f[:],
                ldp[0:CLUSTERS + 1, :, :].rearrange("p g c -> p (g c)"),
                s_ones26, start=True, stop=True)
            s_out = act.tile([GPD * NUM_CLASSES, 1], F32, tag="logits")
            nc.vector.tensor_copy(s_out[:], p_f[:])
            nc.sync.dma_start(
                out=outd[:].rearrange("(p one) -> p one", one=1), in_=s_out[:])

    nc.compile()
    return nc


def make_in_maps(x, a, W1, b1, W2, b2, Wa, ba, Wp, bp, Wc, bc):
    import ml_dtypes
    npmm = np.dtype(ml_dtypes.bfloat16)

    x = np.ascontiguousarray(np.asarray(x, dtype=np.float32))
    a = np.asarray(a, dtype=np.float32)

    ab = a.reshape(B_GRAPHS, NPG, B_GRAPHS, NPG)
    blocks = ab[np.arange(B_GRAPHS), :, np.arange(B_GRAPHS), :].copy()
    blocks[:, np.arange(NPG), np.arange(NPG)] += 1.0    # A + I
    blocks = blocks.astype(npmm)

    wpk = np.zeros((N_FEAT, WP_COLS), npmm)
    wpk[:, WP_W1:WP_W1 + HIDDEN] = np.asarray(W1, np.float32).astype(npmm)
    wpk[0:HIDDEN, WP_W2A:WP_W2A + HIDDEN] = np.asarray(W2, np.float32).astype(npmm)
    wpk[HIDDEN, WP_W2A:WP_W2A + HIDDEN] = np.asarray(b2, np.float32).astype(npmm)
    wpk[0:HIDDEN, WP_WAA:WP_WAA + CLUSTERS] = np.asarray(Wa, np.float32).astype(npmm)
    wpk[HIDDEN, WP_WAA:WP_WAA + CLUSTERS] = np.asarray(ba, np.float32).astype(npmm)
    wpk[0:HIDDEN, WP_WP:WP_WP + HIDDEN] = np.asarray(Wp, np.float32).astype(npmm)
    wpk[0:HIDDEN, WP_WC:WP_WC + NUM_CLASSES] = np.asarray(Wc, np.float32).astype(npmm)
    wpk[HIDDEN + 1, WP_W2A:WP_W2A + HIDDEN] = np.asarray(b1, np.float32).astype(npmm)
    wpk[HIDDEN + 2, WP_W2A:WP_W2A + HIDDEN] = np.asarray(bp, np.float32).astype(npmm)

    fpk = np.zeros((N_FEAT, FP_COLS), np.float32)
    fpk[0:CLUSTERS + 1, FP_ONES] = 1.0
    fpk[0, FP_BC:FP_BC + GPD * NUM_CLASSES] = np.tile(
        np.asarray(bc, np.float32), GPD)

    common = dict(wpk=wpk, fpk=fpk)

    in_maps = []
    for d in range(DEV):
        xd = x[d * GPD * NPG:(d + 1) * GPD * NPG]
        xTl = np.ascontiguousarray(xd.T).reshape(N_FEAT, GPD, NPG).astype(npmm)
        bd = blocks[d * GPD:(d + 1) * GPD]
        bt = np.ascontiguousarray(bd.transpose(1, 0, 2))
        in_maps.append(dict(
            xTa=np.ascontiguousarray(xTl[:, 0:GPD // 2]),
            xTb=np.ascontiguousarray(xTl[:, GPD // 2:]),
            a0a=np.ascontiguousarray(bt[:C0, 0:GPD // 2]),
            a0b=np.ascontiguousarray(bt[:C0, GPD // 2:]),
            a1=np.ascontiguousarray(bt[C0:]),
            **common,
        ))
    return in_maps


def kernel(x, a, seg_ids, num_graphs, W1, b1, W2, b2, Wa, ba, Wp, bp, Wc, bc,
           trace=False):
    if "nc" not in _CACHE:
        _CACHE["nc"] = build_nc()
    nc = _CACHE["nc"]
    in_maps = make_in_maps(x, a, W1, b1, W2, b2, Wa, ba, Wp, bp, Wc, bc)
    res = run_bass_kernel_spmd(nc, in_maps, core_ids=list(range(DEV)), trace=trace)
    logits = np.concatenate(
        [r["out"].reshape(GPD, NUM_CLASSES) for r in res.results], axis=0)
    if trace:
        return logits, res
    return logits


# revision 27
# speedup vs baseline: 1.0990x; 1.0990x over previous
"""GCN + DiffPool kernel for Trainium2, data-parallel over graphs across 8 NeuronCores.

Model (per graph, n=150 nodes):
  Z1 = relu(An @ (x @ W1) + b1)          An = D^-1/2 (A+I) D^-1/2
  Z2 = relu(An @ (Z1 @ W2) + b2)
  S  = softmax(An @ (Z2 @ Wa) + ba)      [n, 25]
  Zp = S^T @ Z2 ; Ap = S^T @ (A @ S)
  H  = relu(Anp @ (Zp @ Wp) + bp)        pooled GCN, 25 cluster-nodes
  logits = (sum_rows H) @ Wc + bc

Sharding: 64 graphs -> 8 devices x 8 graphs; each device gets its graphs'
150x150 diagonal blocks of A+I (node chunks c0=[0:128], c1=[128:150] on
partitions) and node rows of x (feature-major). Final [8,10] logits per
device concatenate on host.

All-node-major dataflow. Every activation keeps nodes on partitions, so the
row normalization factor d = rsqrt(deg+1) is a per-partition scale:
    Z1d = d_j * Z1[j,:] = relu(d_j^2 * psum)         (d>0 commutes with relu)
with psum = sum_i Ah[i,j] (d_i M1[i,h]) + dinv_j*b1[h]; the bias rides an
augmented contraction row (Ah row 22 = dinv_j, M1d row 22 = b1), and the
column factor d_j^2 = 1/(deg_j+1) is applied at the PSUM drain. The
propagate matmuls use lhsT = Ah[i, j-slice] directly (A+I is symmetric), so
no An matrix and no transposes are ever materialized. AS = A@S is recovered
from (A+I)@S - S. The pooled stage folds dp past relu into the readout:
  G @ Wc = sum_c' dp_c' relu(psum_h)[:,c'] @ Wc  ->  per-cluster matmul
then a ones-contraction matmul collapses clusters, with bc on an aug row.
"""

import numpy as np

import concourse.bass as bass
import concourse.mybir as mybir
import concourse.tile as tile
from concourse import bacc
from concourse.bass_utils import run_bass_kernel_spmd

F32 = mybir.dt.float32
BF16 = mybir.dt.bfloat16
U32 = mybir.dt.uint32
AF = mybir.ActivationFunctionType
AL = mybir.AluOpType

MMDT = BF16

N_NODES = 9600
N_FEAT = 128
HIDDEN = 64
CLUSTERS = 25
NUM_CLASSES = 10
B_GRAPHS = 64
NPG = 150            # nodes per graph
DEV = 8              # devices
GPD = 8              # graphs per device
C0, C1 = 128, 22     # node partition chunks

# wpk (bf16) column offsets
WP_W1 = 0                       # [128, 64]
WP_W2A = WP_W1 + HIDDEN         # [65, 64]  row 64 = b2; row 65 = b1; row 66 = bp
WP_WAA = WP_W2A + HIDDEN        # [65, 25]  row 64 = ba
WP_WP = WP_WAA + CLUSTERS       # [64, 64]
WP_WC = WP_WP + HIDDEN          # [64, 10]
WP_COLS = WP_WC + NUM_CLASSES

# fpk (fp32) column offsets
FP_ONES = 0                     # rows 0:26 = 1.0 (ones contraction col)
FP_BC = 1                       # row 0, cols 1:81 = tile(bc, 8)
FP_BP64 = FP_BC + GPD * NUM_CLASSES   # row 0: bp [1, 64]
FP_COLS = FP_BP64 + HIDDEN

_CACHE = {}


def _chunk(c):
    return (0, C0) if c == 0 else (C0, C1)


def build_nc():
    nc = bacc.Bacc("TRN2", target_bir_lowering=False, debug=False, num_devices=DEV)

    def din(name, shape, dt=MMDT):
        return nc.dram_tensor(name, shape, dt, kind="ExternalInput").ap()

    xTa = din("xTa", [N_FEAT, GPD // 2, NPG])
    xTb = din("xTb", [N_FEAT, GPD // 2, NPG])
    a0a = din("a0a", [C0, GPD // 2, NPG])   # rows 0:128 of A+I, graphs 0:4
    a0b = din("a0b", [C0, GPD // 2, NPG])   # rows 0:128 of A+I, graphs 4:8
    a1 = din("a1", [C1, GPD, NPG])          # rows 128:150 of A+I blocks
    wpk = din("wpk", [N_FEAT, WP_COLS])
    fpk = din("fpk", [N_FEAT, FP_COLS], F32)
    outd = nc.dram_tensor("out", [GPD, NUM_CLASSES], F32, kind="ExternalOutput").ap()

    with tile.TileContext(nc) as tc:
        with (
            tc.tile_pool(name="cst", bufs=1) as cst,
            tc.tile_pool(name="act", bufs=1) as act,
            tc.tile_pool(name="ps", bufs=3, space="PSUM") as ps,
            tc.tile_pool(name="psu", bufs=3, space="PSUM") as psu,
            tc.tile_pool(name="pst", bufs=1, space="PSUM") as pst,
            tc.tile_pool(name="dram", bufs=1, space="DRAM") as dram,
        ):
            H2 = GPD // 2

            # ---- PE warmup + input DMAs. Only the gpsimd SWDGE queue moves
            # big multi-row transfers at full rate; the adjacency (heads the
            # degree->d->aug critical chain) goes there first. sync HWDGE
            # takes the rest; scalar stays free for ACT work.
            warm = cst.tile([C0, 256], MMDT, tag="warm")
            nc.vector.memset(warm[:], 1.0)
            pwt = ps.tile([C0, 512], F32, tag="ps")
            for _ in range(12):
                nc.tensor.matmul(pwt[:, 0:256], warm[:, 0:C0], warm[:],
                                 start=True, stop=True)

            s_a1 = cst.tile([C1 + 1, GPD, NPG], MMDT, tag="a1")
            nc.gpsimd.dma_start(out=s_a1[0:C1, :, :], in_=a1)
            s_a0 = cst.tile([C0, GPD, NPG], MMDT, tag="a0")
            nc.gpsimd.dma_start(out=s_a0[:, 0:H2, :], in_=a0a)
            nc.gpsimd.dma_start(out=s_a0[:, H2:GPD, :], in_=a0b)
            s_xT = cst.tile([N_FEAT, GPD, NPG], MMDT, tag="xT")
            nc.gpsimd.dma_start(out=s_xT[:, H2:GPD, :], in_=xTb)
            s_wpk = cst.tile([N_FEAT, WP_COLS], MMDT, tag="wpk")
            nc.sync.dma_start(out=s_wpk[:], in_=wpk)
            nc.sync.dma_start(out=s_xT[:, 0:H2, :], in_=xTa)
            s_fpk = cst.tile([N_FEAT, FP_COLS], F32, tag="fpk")
            nc.sync.dma_start(out=s_fpk[:], in_=fpk)

            # identity built on device (saves a 32KB const load)
            idt = cst.tile([C0, C0], MMDT, tag="idt")
            nc.gpsimd.affine_select(idt[:], warm[:, 0:C0], [[1, C0]],
                                    AL.is_equal, 0.0, base=0,
                                    channel_multiplier=-1)

            s_a = (s_a0, s_a1)
            s_W1 = s_wpk[:, WP_W1:WP_W1 + HIDDEN]
            s_W2a = s_wpk[0:HIDDEN + 1, WP_W2A:WP_W2A + HIDDEN]
            s_Waa = s_wpk[0:HIDDEN + 1, WP_WAA:WP_WAA + CLUSTERS]
            s_Wp = s_wpk[0:HIDDEN, WP_WP:WP_WP + HIDDEN]
            s_Wc = s_wpk[0:HIDDEN, WP_WC:WP_WC + NUM_CLASSES]
            s_ones26 = s_fpk[0:CLUSTERS + 1, FP_ONES:FP_ONES + 1]

            # ---- degrees: rowsum(A+I) = deg+1 on partitions ----------------
            degc = act.tile([C0, 2 * GPD], F32, tag="degc")
            nc.vector.memset(degc[0:C0, GPD:2 * GPD], 1.0)
            nc.vector.reduce_sum(out=degc[0:C1, GPD:2 * GPD], in_=s_a1[0:C1, :, :],
                                 axis=mybir.AxisListType.X)
            nc.vector.reduce_sum(out=degc[:, 0:H2], in_=s_a0[:, 0:H2, :],
                                 axis=mybir.AxisListType.X)
            rscr = act.tile([C0, NPG], MMDT, tag="rscr")
            for g in range(H2, GPD):
                nc.scalar.activation(rscr[:], s_a0[:, g, :], AF.Copy,
                                     accum_out=degc[:, g:g + 1])

            def emit_rsqrt(x, rows, cols):
                """rsqrt via exp(-0.5*ln(x)) on ACT -- ln/exp share one
                table set with the softmax Exp, so no table thrash."""
                lg = act.tile([rows, cols], F32, tag=f"lg_{id(x)}")
                nc.scalar.activation(lg[:], x[:], AF.Ln)
                s = act.tile([rows, cols], F32, tag=f"rs_{id(x)}")
                nc.scalar.activation(s[:], lg[:], AF.Exp, scale=-0.5)
                return s

            dcomb = emit_rsqrt(degc, C0, 2 * GPD)          # d = rsqrt(deg+1)
            d2comb = act.tile([C0, 2 * GPD], F32, tag="d2c")
            nc.vector.reciprocal(d2comb[:], degc[:])       # d^2 = 1/(deg+1)
            dinvc = act.tile([C0, 2 * GPD], F32, tag="dic")
            nc.vector.tensor_mul(dinvc[:], dcomb[:], degc[:])   # 1/d = sqrt(deg+1)
            dinvb = act.tile([C0, 2 * GPD], MMDT, tag="dib")
            nc.vector.tensor_copy(dinvb[:], dinvc[:])

            s_d = [dcomb[:, 0:GPD], dcomb[0:C1, GPD:2 * GPD]]
            s_d2 = [d2comb[:, 0:GPD], d2comb[0:C1, GPD:2 * GPD]]
            s_dinvb = [dinvb[:, 0:GPD], dinvb[0:C1, GPD:2 * GPD]]

            # ---- dinv as a free-dim row via PE transpose + DRAM hop --------
            p_dt = pst.tile([GPD * 2, 160], MMDT, tag="ptr")
            nc.tensor.transpose(p_dt[0:GPD, 0:C0], s_dinvb[0][:], idt[:])
            nc.tensor.transpose(p_dt[0:GPD, C0:NPG], s_dinvb[1][:],
                                idt[0:C1, 0:C1])
            dTrow = act.tile([GPD, NPG], MMDT, tag="dTrow")
            nc.vector.tensor_copy(dTrow[:], p_dt[0:GPD, 0:NPG])
            dTd = dram.tile([GPD * NPG], MMDT, tag="dTd")
            nc.sync.dma_start(out=dTd[:].rearrange("(g j) -> g j", g=GPD),
                              in_=dTrow[:])
            dinv_row = dTd[:].rearrange("(g j) -> g j", g=GPD)[None, :, :]
            # aug row 22 of the chunk-1 adjacency: dinv_j
            nc.sync.dma_start(out=s_a1[C1:C1 + 1, :, :], in_=dinv_row)

            # ---- M1 = (X @ W1) * d_row, node-major; aug row = b1 -----------
            m1 = []
            for c, cn in ((0, C0), (1, C1)):
                off, _ = _chunk(c)
                p = ps.tile([C0, GPD, HIDDEN], F32, tag="ps")
                for g in range(GPD):
                    nc.tensor.matmul(p[0:cn, g, :],
                                     s_xT[:, g, off:off + cn], s_W1,
                                     start=True, stop=True)
                rows = cn + (1 if c == 1 else 0)
                o = act.tile([rows, GPD, HIDDEN], MMDT, tag=f"m1_{c}")
                dbc = s_d[c][:][:, :, None].broadcast_to((cn, GPD, HIDDEN))
                nc.vector.tensor_mul(o[0:cn, :, :], p[0:cn, :, :], dbc)
                if c == 1:
                    b1b = wpk[HIDDEN + 1:HIDDEN + 2, WP_W2A:WP_W2A + HIDDEN] \
                        [:, None, :].broadcast_to((1, GPD, HIDDEN))
                    nc.gpsimd.dma_start(out=o[C1:C1 + 1, :, :], in_=b1b)
                m1.append(o)

            def prop_nm(rhs_tiles, d2s, name, fout=HIDDEN):
                """Z[j,h] = relu(d_j^2 * sum_i Ah_aug[i,j] rhs_aug[i,h]).
                rhs tiles: (c0 [128,g,fout], c1 [23,g,fout] w/ aug row)."""
                outs = []
                ptiles = []
                for jc, jn in ((0, C0), (1, C1)):
                    joff, _ = _chunk(jc)
                    p = ps.tile([C0, GPD, fout], F32, tag="ps")
                    ptiles.append(p)
                    for g in range(GPD):
                        nc.tensor.matmul(p[0:jn, g, :],
                                         s_a0[:, g, joff:joff + jn],
                                         rhs_tiles[0][0:C0, g, :],
                                         start=True, stop=False)
                        nc.tensor.matmul(p[0:jn, g, :],
                                         s_a1[0:C1 + 1, g, joff:joff + jn],
                                         rhs_tiles[1][0:C1 + 1, g, :],
                                         start=False, stop=True)
                for jc, jn in ((0, C0), (1, C1)):
                    p = ptiles[jc]
                    o = act.tile([jn, GPD, fout], MMDT, tag=f"{name}{jc}")
                    d2bc = d2s[jc][:][:, :, None].broadcast_to((jn, GPD, fout))
                    nc.vector.scalar_tensor_tensor(
                        o[:], p[0:jn, :, :], 0.0, d2bc, AL.max, AL.mult)
                    outs.append(o)
                return outs

            # ---- layer 1: Z1d = d * relu(An@M1 + b1) -----------------------
            z1d = prop_nm(m1, s_d2, "z1d")

            # ---- U = raw(An @ Z1), feature-major; aug row 64 = dinv --------
            def an_prop_fm(lhs_tiles, name):
                o = act.tile([HIDDEN + 1, GPD, NPG], MMDT, tag=name)
                nc.scalar.dma_start(out=o[HIDDEN:HIDDEN + 1, :, :], in_=dinv_row)
                for h in range(0, GPD, 2):
                    p = psu.tile([HIDDEN, 2, 256], F32, tag="psu")
                    for gg in range(2):
                        g = h + gg
                        for c, cn in ((0, C0), (1, C1)):
                            off, _ = _chunk(c)
                            nc.tensor.matmul(p[:, gg, 0:NPG],
                                             lhs_tiles[c][0:cn, g, :],
                                             s_a[c][0:cn, g, :],
                                             start=(c == 0), stop=(c == 1))
                    nc.scalar.copy(o[0:HIDDEN, h:h + 2, :], p[:, :, 0:NPG])
                return o

            u = an_prop_fm(z1d, "u")

            # ---- layer 2: Z2d = d * relu((U@W2)*d + b2) --------------------
            def w_stage_nm(lhs_fm, w_aug, d2s, name, fout=HIDDEN, relu=True):
                """out[j,:] = drain(d_j^2 * sum_h lhs_aug[h,j] w_aug[h,:])."""
                outs = []
                for jc, jn in ((0, C0), (1, C1)):
                    joff, _ = _chunk(jc)
                    p = ps.tile([C0, GPD, fout], F32, tag="ps")
                    for g in range(GPD):
                        nc.tensor.matmul(p[0:jn, g, :],
                                         lhs_fm[0:HIDDEN + 1, g, joff:joff + jn],
                                         w_aug, start=True, stop=True)
                    outs.append(p)
                return outs

            p2 = w_stage_nm(u, s_W2a, s_d2, "p2")
            z2d = []
            for jc, jn in ((0, C0), (1, C1)):
                o = act.tile([jn, GPD, HIDDEN], MMDT, tag=f"z2d{jc}")
                d2bc = s_d2[jc][:][:, :, None].broadcast_to((jn, GPD, HIDDEN))
                nc.vector.scalar_tensor_tensor(
                    o[:], p2[jc][0:jn, :, :], 0.0, d2bc, AL.max, AL.mult)
                z2d.append(o)

            # ---- T = raw(An @ Z2), then P = T@Wa, softmax ------------------
            t = an_prop_fm(z2d, "t")
            pp = w_stage_nm(t, s_Waa, None, "pp", fout=CLUSTERS)

            s_S, s_Si = [], []
            for jc, jn in ((0, C0), (1, C1)):
                pm = act.tile([jn, GPD, CLUSTERS], F32, tag=f"pm{jc}")
                dbc = s_d[jc][:][:, :, None].broadcast_to((jn, GPD, CLUSTERS))
                nc.vector.tensor_mul(pm[:], pp[jc][0:jn, :, :], dbc)
                e = act.tile([jn, GPD, CLUSTERS], F32, tag=f"e{jc}")
                nc.scalar.activation(e[:], pm[:], AF.Exp)
                ssum = act.tile([jn, GPD], F32, tag=f"ssum{jc}")
                nc.vector.reduce_sum(out=ssum[:], in_=e[:],
                                     axis=mybir.AxisListType.X)
                rs = act.tile([jn, GPD], F32, tag=f"rsx{jc}")
                nc.vector.reciprocal(rs[:], ssum[:])
                s = act.tile([jn, GPD, CLUSTERS], MMDT, tag=f"s{jc}")
                nc.vector.tensor_mul(
                    s[:], e[:], rs[:][:, :, None].broadcast_to((jn, GPD, CLUSTERS)))
                s_S.append(s)
                si = act.tile([jn, GPD, CLUSTERS], MMDT, tag=f"si{jc}")
                dib = s_dinvb[jc][:][:, :, None].broadcast_to((jn, GPD, CLUSTERS))
                nc.vector.tensor_mul(si[:], s[:], dib)
                s_Si.append(si)

            # ---- AS = (A+I)@S - S, node-major ------------------------------
            s_AS = []
            for jc, jn in ((0, C0), (1, C1)):
                joff, _ = _chunk(jc)
                p = ps.tile([C0, GPD, CLUSTERS], F32, tag="ps")
                for g in range(GPD):
                    for c, cn in ((0, C0), (1, C1)):
                        nc.tensor.matmul(p[0:jn, g, :],
                                         s_a[c][0:cn, g, joff:joff + jn],
                                         s_S[c][0:cn, g, :],
                                         start=(c == 0), stop=(c == 1))
                o = act.tile([jn, GPD, CLUSTERS], MMDT, tag=f"as{jc}")
                nc.vector.tensor_sub(o[:], p[0:jn, :, :], s_S[jc][:])
                s_AS.append(o)

            # ---- Ap = S^T @ AS ; Zp^T = Z2^T @ S ---------------------------
            p_ap = ps.tile([CLUSTERS, GPD, CLUSTERS], F32, tag="ps")
            for g in range(GPD):
                for c, cn in ((0, C0), (1, C1)):
                    nc.tensor.matmul(p_ap[:, g, :], s_S[c][0:cn, g, :],
                                     s_AS[c][0:cn, g, :],
                                     start=(c == 0), stop=(c == 1))
            p_zp = ps.tile([HIDDEN, GPD, CLUSTERS], F32, tag="ps")
            for g in range(GPD):
                for c, cn in ((0, C0), (1, C1)):
                    nc.tensor.matmul(p_zp[:, g, :], z2d[c][0:cn, g, :],
                                     s_Si[c][0:cn, g, :],
                                     start=(c == 0), stop=(c == 1))
            s_Zp = act.tile([HIDDEN, GPD, CLUSTERS], MMDT, tag="zp")
            nc.scalar.copy(s_Zp[:], p_zp[:])

            # ---- pooled normalization --------------------------------------
            degp = act.tile([CLUSTERS, GPD], F32, tag="degp")
            nc.vector.reduce_sum(out=degp[:], in_=p_ap[:],
                                 axis=mybir.AxisListType.X)
            nc.vector.tensor_scalar_add(degp[:], degp[:], 1.0)
            dp = emit_rsqrt(degp, CLUSTERS, GPD)

            # Ahp = Ap + I (node-major pooled stage: both dp factors are
            # partition scales, so no transposed dinvp row is ever needed)
            ahp = act.tile([CLUSTERS, GPD, CLUSTERS], MMDT, tag="ahp")
            id25b = idt[0:CLUSTERS, 0:CLUSTERS][:, None, :] \
                .broadcast_to((CLUSTERS, GPD, CLUSTERS))
            nc.vector.tensor_add(ahp[:], p_ap[:], id25b)

            # bp broadcast to [25, g, 64] (const, loads early)
            bp64 = cst.tile([CLUSTERS, GPD, HIDDEN], MMDT, tag="bp64")
            bpb = wpk[HIDDEN + 2:HIDDEN + 3, WP_W2A:WP_W2A + HIDDEN] \
                [:, None, :].broadcast_to((CLUSTERS, GPD, HIDDEN))
            nc.scalar.dma_start(out=bp64[:], in_=bpb)
            bc8 = cst.tile([GPD, NUM_CLASSES], F32, tag="bc8")
            nc.scalar.dma_start(
                out=bc8[:],
                in_=fpk[0:1, FP_BC:FP_BC + GPD * NUM_CLASSES]
                .rearrange("one (g c) -> (one g) c", g=GPD))

            # ---- pooled GCN: H^T = relu(dp_c' (Ahp^T ZWd) + bp), node-major
            p_zw = ps.tile([CLUSTERS, GPD, HIDDEN], F32, tag="ps")
            for g in range(GPD):
                nc.tensor.matmul(p_zw[:, g, :], s_Zp[:, g, :], s_Wp,
                                 start=True, stop=True)
            zwd = act.tile([CLUSTERS, GPD, HIDDEN], MMDT, tag="zwd")
            dpbc = dp[:][:, :, None].broadcast_to((CLUSTERS, GPD, HIDDEN))
            nc.vector.tensor_mul(zwd[:], p_zw[:], dpbc)

            p_h = ps.tile([CLUSTERS, GPD, HIDDEN], F32, tag="ps")
            for g in range(GPD):
                nc.tensor.matmul(p_h[:, g, :], ahp[:, g, :], zwd[:, g, :],
                                 start=True, stop=True)
            th = act.tile([CLUSTERS, GPD, HIDDEN], MMDT, tag="th")
            nc.vector.tensor_mul(th[:], p_h[:], dpbc)
            y2 = act.tile([CLUSTERS, GPD, HIDDEN], MMDT, tag="y2")
            nc.vector.tensor_add(y2[:], th[:], bp64[:])
            y = act.tile([CLUSTERS, GPD, HIDDEN], MMDT, tag="y")
            nc.vector.tensor_scalar_max(y[:], y2[:], 0.0)

            # ---- readout: G = sum_c' Y[c',:], logits = G @ Wc + bc ---------
            p_g = ps.tile([HIDDEN, GPD, 1], F32, tag="ps")
            for g in range(GPD):
                nc.tensor.matmul(p_g[:, g, :], y[:, g, :],
                                 warm[0:CLUSTERS, 0:1],
                                 start=True, stop=True)
            gb = act.tile([HIDDEN, GPD], MMDT, tag="gb")
            nc.scalar.copy(gb[:], p_g[:, :, 0])

            p_f = pst.tile([GPD, NUM_CLASSES], F32, tag="pf")
            nc.tensor.matmul(p_f[:], gb[:], s_Wc, start=True, stop=True)
            s_out = act.tile([GPD, NUM_CLASSES], F32, tag="logits")
            nc.vector.tensor_add(s_out[:], p_f[:], bc8[:])
            nc.sync.dma_start(out=outd, in_=s_out[:])

    nc.compile()
    return nc


def make_in_maps(x, a, W1, b1, W2, b2, Wa, ba, Wp, bp, Wc, bc):
    import ml_dtypes
    npmm = np.dtype(ml_dtypes.bfloat16)

    x = np.ascontiguousarray(np.asarray(x, dtype=np.float32))
    a = np.asarray(a, dtype=np.float32)

    ab = a.reshape(B_GRAPHS, NPG, B_GRAPHS, NPG)
    blocks = ab[np.arange(B_GRAPHS), :, np.arange(B_GRAPHS), :].copy()
    blocks[:, np.arange(NPG), np.arange(NPG)] += 1.0    # A + I
    blocks = blocks.astype(npmm)

    wpk = np.zeros((N_FEAT, WP_COLS), npmm)
    wpk[:, WP_W1:WP_W1 + HIDDEN] = np.asarray(W1, np.float32).astype(npmm)
    wpk[0:HIDDEN, WP_W2A:WP_W2A + HIDDEN] = np.asarray(W2, np.float32).astype(npmm)
    wpk[HIDDEN, WP_W2A:WP_W2A + HIDDEN] = np.asarray(b2, np.float32).astype(npmm)
    wpk[0:HIDDEN, WP_WAA:WP_WAA + CLUSTERS] = np.asarray(Wa, np.float32).astype(npmm)
    wpk[HIDDEN, WP_WAA:WP_WAA + CLUSTERS] = np.asarray(ba, np.float32).astype(npmm)
    wpk[0:HIDDEN, WP_WP:WP_WP + HIDDEN] = np.asarray(Wp, np.float32).astype(npmm)
    wpk[0:HIDDEN, WP_WC:WP_WC + NUM_CLASSES] = np.asarray(Wc, np.float32).astype(npmm)
    wpk[HIDDEN + 1, WP_W2A:WP_W2A + HIDDEN] = np.asarray(b1, np.float32).astype(npmm)
    wpk[HIDDEN + 2, WP_W2A:WP_W2A + HIDDEN] = np.asarray(bp, np.float32).astype(npmm)

    fpk = np.zeros((N_FEAT, FP_COLS), np.float32)
    fpk[0:CLUSTERS + 1, FP_ONES] = 1.0
    fpk[0, FP_BC:FP_BC + GPD * NUM_CLASSES] = np.tile(
        np.asarray(bc, np.float32), GPD)
    fpk[0, FP_BP64:FP_BP64 + HIDDEN] = np.asarray(bp, np.float32)

    common = dict(wpk=wpk, fpk=fpk)

    in_maps = []
    for d in range(DEV):
        xd = x[d * GPD * NPG:(d + 1) * GPD * NPG]
        xTl = np.ascontiguousarray(xd.T).reshape(N_FEAT, GPD, NPG).astype(npmm)
        bd = blocks[d * GPD:(d + 1) * GPD]
        bt = np.ascontiguousarray(bd.transpose(1, 0, 2))
        in_maps.append(dict(
            xTa=np.ascontiguousarray(xTl[:, 0:GPD // 2]),
            xTb=np.ascontiguousarray(xTl[:, GPD // 2:]),
            a0a=np.ascontiguousarray(bt[:C0, 0:GPD // 2]),
            a0b=np.ascontiguousarray(bt[:C0, GPD // 2:]),
            a1=np.ascontiguousarray(bt[C0:]),
            **common,
        ))
    return in_maps


def kernel(x, a, seg_ids, num_graphs, W1, b1, W2, b2, Wa, ba, Wp, bp, Wc, bc,
           trace=False):
    if "nc" not in _CACHE:
        _CACHE["nc"] = build_nc()
    nc = _CACHE["nc"]
    in_maps = make_in_maps(x, a, W1, b1, W2, b2, Wa, ba, Wp, bp, Wc, bc)
    res = run_bass_kernel_spmd(nc, in_maps, core_ids=list(range(DEV)), trace=trace)
    logits = np.concatenate(
        [r["out"] for r in res.results], axis=0)
    if trace:
        return logits, res
    return logits


# revision 28
# speedup vs baseline: 1.2161x; 1.1066x over previous
"""GCN + DiffPool kernel for Trainium2, data-parallel over graphs across 8 NeuronCores.

Model (per graph, n=150 nodes):
  Z1 = relu(An @ (x @ W1) + b1)          An = D^-1/2 (A+I) D^-1/2
  Z2 = relu(An @ (Z1 @ W2) + b2)
  S  = softmax(An @ (Z2 @ Wa) + ba)      [n, 25]
  Zp = S^T @ Z2 ; Ap = S^T @ (A @ S)
  H  = relu(Anp @ (Zp @ Wp) + bp)        pooled GCN, 25 cluster-nodes
  logits = (sum_rows H) @ Wc + bc

Sharding: 64 graphs -> 8 devices x 8 graphs; each device gets its graphs'
150x150 diagonal blocks of A+I (node chunks c0=[0:128], c1=[128:150] on
partitions) and node rows of x (feature-major). Final [8,10] logits per
device concatenate on host.

All-node-major dataflow. Every activation keeps nodes on partitions, so the
row normalization factor d = rsqrt(deg+1) is a per-partition scale:
    Z1d = d_j * Z1[j,:] = relu(d_j^2 * psum)         (d>0 commutes with relu)
with psum = sum_i Ah[i,j] (d_i M1[i,h]) + dinv_j*b1[h]; the bias rides an
augmented contraction row (Ah row 22 = dinv_j, M1d row 22 = b1), and the
column factor d_j^2 = 1/(deg_j+1) is applied at the PSUM drain. The
propagate matmuls use lhsT = Ah[i, j-slice] directly (A+I is symmetric), so
no An matrix and no transposes are ever materialized. AS = A@S is recovered
from (A+I)@S - S. The pooled stage folds dp past relu into the readout:
  G @ Wc = sum_c' dp_c' relu(psum_h)[:,c'] @ Wc  ->  per-cluster matmul
then a ones-contraction matmul collapses clusters, with bc on an aug row.
"""

import numpy as np

import concourse.bass as bass
import concourse.mybir as mybir
import concourse.tile as tile
from concourse import bacc
from concourse.bass_utils import run_bass_kernel_spmd

F32 = mybir.dt.float32
BF16 = mybir.dt.bfloat16
U32 = mybir.dt.uint32
AF = mybir.ActivationFunctionType
AL = mybir.AluOpType

MMDT = BF16

N_NODES = 9600
N_FEAT = 128
HIDDEN = 64
CLUSTERS = 25
NUM_CLASSES = 10
B_GRAPHS = 64
NPG = 150            # nodes per graph
DEV = 8              # devices
GPD = 8              # graphs per device
C0, C1 = 128, 22     # node partition chunks

# wpk (bf16) column offsets
WP_W1 = 0                       # [128, 64]
WP_W2A = WP_W1 + HIDDEN         # [65, 64]  row 64 = b2; row 65 = b1; row 66 = bp
WP_WAA = WP_W2A + HIDDEN        # [65, 25]  row 64 = ba
WP_WP = WP_WAA + CLUSTERS       # [64, 64]
WP_WC = WP_WP + HIDDEN          # [64, 10]
WP_COLS = WP_WC + NUM_CLASSES

# fpk (fp32) column offsets
FP_ONES = 0                     # rows 0:26 = 1.0 (ones contraction col)
FP_BC = 1                       # row 0, cols 1:81 = tile(bc, 8)
FP_BP64 = FP_BC + GPD * NUM_CLASSES   # row 0: bp [1, 64]
FP_COLS = FP_BP64 + HIDDEN

_CACHE = {}


def _chunk(c):
    return (0, C0) if c == 0 else (C0, C1)


def build_nc():
    nc = bacc.Bacc("TRN2", target_bir_lowering=False, debug=False, num_devices=DEV)

    def din(name, shape, dt=MMDT):
        return nc.dram_tensor(name, shape, dt, kind="ExternalInput").ap()

    xTa = din("xTa", [N_FEAT, GPD // 2, NPG])
    xTb = din("xTb", [N_FEAT, GPD // 2, NPG])
    FP8 = mybir.dt.float8e4
    a0a = din("a0a", [C0, GPD // 2, NPG], FP8)   # A+I rows 0:128, graphs 0:4
    a0b = din("a0b", [C0, GPD // 2, NPG], FP8)   # A+I rows 0:128, graphs 4:8
    a1 = din("a1", [C1, GPD, NPG], FP8)          # A+I rows 128:150
    wpk = din("wpk", [N_FEAT, WP_COLS])
    fpk = din("fpk", [32, FP_COLS], F32)
    outd = nc.dram_tensor("out", [GPD, NUM_CLASSES], F32, kind="ExternalOutput").ap()

    with tile.TileContext(nc) as tc:
        with (
            tc.tile_pool(name="cst", bufs=1) as cst,
            tc.tile_pool(name="act", bufs=1) as act,
            tc.tile_pool(name="ps", bufs=3, space="PSUM") as ps,
            tc.tile_pool(name="psu", bufs=3, space="PSUM") as psu,
            tc.tile_pool(name="pst", bufs=1, space="PSUM") as pst,
            tc.tile_pool(name="dram", bufs=1, space="DRAM") as dram,
        ):
            H2 = GPD // 2

            # ---- PE warmup + input DMAs. Only the gpsimd SWDGE queue moves
            # big multi-row transfers at full rate; the adjacency (heads the
            # degree->d->aug critical chain) goes there first. sync HWDGE
            # takes the rest; scalar stays free for ACT work.
            warm = cst.tile([C0, 256], MMDT, tag="warm")
            nc.vector.memset(warm[:], 1.0)
            pwt = ps.tile([C0, 512], F32, tag="ps")
            for _ in range(12):
                nc.tensor.matmul(pwt[:, 0:256], warm[:, 0:C0], warm[:],
                                 start=True, stop=True)

            s_a1 = cst.tile([C1 + 1, GPD, NPG], MMDT, tag="a1")
            nc.gpsimd.dma_start(out=s_a1[0:C1, :, :], in_=a1)
            s_a0 = cst.tile([C0, GPD, NPG], MMDT, tag="a0")
            nc.gpsimd.dma_start(out=s_a0[:, 0:H2, :], in_=a0a)
            nc.gpsimd.dma_start(out=s_a0[:, H2:GPD, :], in_=a0b)
            s_wpk = cst.tile([N_FEAT, WP_COLS], MMDT, tag="wpk")
            nc.sync.dma_start(out=s_wpk[:], in_=wpk)
            s_xT = cst.tile([N_FEAT, GPD, NPG], MMDT, tag="xT")
            nc.scalar.dma_start(out=s_xT[:, H2:GPD, :], in_=xTb)
            nc.sync.dma_start(out=s_xT[:, 0:H2, :], in_=xTa)
            s_fpk = cst.tile([32, FP_COLS], F32, tag="fpk")
            nc.scalar.dma_start(out=s_fpk[:], in_=fpk)

            # identity built on device (saves a 32KB const load)
            idt = cst.tile([C0, C0], MMDT, tag="idt")
            nc.gpsimd.affine_select(idt[:], warm[:, 0:C0], [[1, C0]],
                                    AL.is_equal, 0.0, base=0,
                                    channel_multiplier=-1)

            s_a = (s_a0, s_a1)
            s_W1 = s_wpk[:, WP_W1:WP_W1 + HIDDEN]
            s_W2a = s_wpk[0:HIDDEN + 1, WP_W2A:WP_W2A + HIDDEN]
            s_Waa = s_wpk[0:HIDDEN + 1, WP_WAA:WP_WAA + CLUSTERS]
            s_Wp = s_wpk[0:HIDDEN, WP_WP:WP_WP + HIDDEN]
            s_Wc = s_wpk[0:HIDDEN, WP_WC:WP_WC + NUM_CLASSES]
            s_ones26 = s_fpk[0:CLUSTERS + 1, FP_ONES:FP_ONES + 1]

            # ---- degrees: rowsum(A+I) = deg+1 on partitions ----------------
            degc = act.tile([C0, 2 * GPD], F32, tag="degc")
            nc.vector.memset(degc[0:C0, GPD:2 * GPD], 1.0)
            nc.vector.reduce_sum(out=degc[0:C1, GPD:2 * GPD], in_=s_a1[0:C1, :, :],
                                 axis=mybir.AxisListType.X)
            nc.vector.reduce_sum(out=degc[:, 0:H2], in_=s_a0[:, 0:H2, :],
                                 axis=mybir.AxisListType.X)
            nc.vector.reduce_sum(out=degc[:, H2:GPD], in_=s_a0[:, H2:GPD, :],
                                 axis=mybir.AxisListType.X)

            qk1 = act.tile([C0, 1], U32, tag="qk1")
            nc.vector.memset(qk1[:], 1)
            qkm = act.tile([C0, 1], U32, tag="qkm")
            nc.vector.memset(qkm[:], 0x5F3759DF)

            def emit_rsqrt(x, rows, cols, iters=2):
                s = act.tile([rows, cols], F32, tag=f"rs_{id(x)}")
                w = act.tile([rows, cols], F32, tag=f"rw_{id(x)}")
                nc.vector.tensor_tensor(s[:].bitcast(U32), x[:].bitcast(U32),
                                        qk1[0:rows, :].broadcast_to((rows, cols)),
                                        AL.logical_shift_right)
                nc.vector.tensor_tensor(s[:].bitcast(U32),
                                        qkm[0:rows, :].broadcast_to((rows, cols)),
                                        s[:].bitcast(U32), AL.subtract)
                for _ in range(iters):
                    nc.vector.tensor_mul(w[:], s[:], s[:])
                    nc.vector.tensor_mul(w[:], w[:], x[:])
                    nc.vector.tensor_scalar(w[:], w[:], -0.5, 1.5, AL.mult, AL.add)
                    nc.vector.tensor_mul(s[:], s[:], w[:])
                return s

            dcomb = emit_rsqrt(degc, C0, 2 * GPD)          # d = rsqrt(deg+1)
            dbcomb = act.tile([C0, 2 * GPD], MMDT, tag="dbc")
            nc.vector.tensor_copy(dbcomb[:], dcomb[:])
            d2comb = act.tile([C0, 2 * GPD], F32, tag="d2c")
            nc.vector.reciprocal(d2comb[:], degc[:])       # d^2 = 1/(deg+1)
            dinvc = act.tile([C0, 2 * GPD], F32, tag="dic")
            nc.vector.tensor_mul(dinvc[:], dcomb[:], degc[:])   # 1/d = sqrt(deg+1)
            dinvb = act.tile([C0, 2 * GPD], MMDT, tag="dib")
            nc.vector.tensor_copy(dinvb[:], dinvc[:])

            s_d = [dcomb[:, 0:GPD], dcomb[0:C1, GPD:2 * GPD]]
            s_db = [dbcomb[:, 0:GPD], dbcomb[0:C1, GPD:2 * GPD]]
            s_d2 = [d2comb[:, 0:GPD], d2comb[0:C1, GPD:2 * GPD]]
            s_dinvb = [dinvb[:, 0:GPD], dinvb[0:C1, GPD:2 * GPD]]

            # ---- dinv as a free-dim row via PE transpose + DRAM hop --------
            p_dt = pst.tile([GPD * 2, 160], MMDT, tag="ptr")
            nc.tensor.transpose(p_dt[0:GPD, 0:C0], s_dinvb[0][:], idt[:])
            nc.tensor.transpose(p_dt[0:GPD, C0:NPG], s_dinvb[1][:],
                                idt[0:C1, 0:C1])
            dTrow = act.tile([GPD, NPG], MMDT, tag="dTrow")
            nc.vector.tensor_copy(dTrow[:], p_dt[0:GPD, 0:NPG])
            dTd = dram.tile([GPD * NPG], MMDT, tag="dTd")
            nc.sync.dma_start(out=dTd[:].rearrange("(g j) -> g j", g=GPD),
                              in_=dTrow[:])
            dinv_row = dTd[:].rearrange("(g j) -> g j", g=GPD)[None, :, :]
            # aug row 22 of the chunk-1 adjacency: dinv_j
            nc.sync.dma_start(out=s_a1[C1:C1 + 1, :, :], in_=dinv_row)

            # ---- M1 = (X @ W1) * d_row, node-major; aug row = b1 -----------
            m1 = []
            for c, cn in ((0, C0), (1, C1)):
                off, _ = _chunk(c)
                p = ps.tile([C0, GPD, HIDDEN], F32, tag="ps")
                for g in range(GPD):
                    nc.tensor.matmul(p[0:cn, g, :],
                                     s_xT[:, g, off:off + cn], s_W1,
                                     start=True, stop=True)
                rows = cn + (1 if c == 1 else 0)
                mr = act.tile([C0, GPD, HIDDEN], MMDT, tag=f"m1r_{c}")
                nc.scalar.copy(mr[0:cn, :, :], p[0:cn, :, :])
                o = act.tile([rows, GPD, HIDDEN], MMDT, tag=f"m1_{c}")
                dbb = s_db[c][:][:, :, None].broadcast_to((cn, GPD, HIDDEN))
                nc.gpsimd.tensor_mul(o[0:cn, :, :], mr[0:cn, :, :], dbb)
                if c == 1:
                    b1b = wpk[HIDDEN + 1:HIDDEN + 2, WP_W2A:WP_W2A + HIDDEN] \
                        [:, None, :].broadcast_to((1, GPD, HIDDEN))
                    nc.gpsimd.dma_start(out=o[C1:C1 + 1, :, :], in_=b1b)
                m1.append(o)

            def prop_nm(rhs_tiles, d2s, name, fout=HIDDEN):
                """Z[j,h] = relu(d_j^2 * sum_i Ah_aug[i,j] rhs_aug[i,h]).
                rhs tiles: (c0 [128,g,fout], c1 [23,g,fout] w/ aug row)."""
                outs = []
                ptiles = []
                for jc, jn in ((0, C0), (1, C1)):
                    joff, _ = _chunk(jc)
                    p = ps.tile([C0, GPD, fout], F32, tag="ps")
                    ptiles.append(p)
                    for g in range(GPD):
                        nc.tensor.matmul(p[0:jn, g, :],
                                         s_a0[:, g, joff:joff + jn],
                                         rhs_tiles[0][0:C0, g, :],
                                         start=True, stop=False)
                        nc.tensor.matmul(p[0:jn, g, :],
                                         s_a1[0:C1 + 1, g, joff:joff + jn],
                                         rhs_tiles[1][0:C1 + 1, g, :],
                                         start=False, stop=True)
                for jc, jn in ((0, C0), (1, C1)):
                    p = ptiles[jc]
                    o = act.tile([jn, GPD, fout], MMDT, tag=f"{name}{jc}")
                    d2bc = d2s[jc][:][:, :, None].broadcast_to((jn, GPD, fout))
                    nc.vector.scalar_tensor_tensor(
                        o[:], p[0:jn, :, :], 0.0, d2bc, AL.max, AL.mult)
                    outs.append(o)
                return outs

            # ---- layer 1: Z1d = d * relu(An@M1 + b1) -----------------------
            z1d = prop_nm(m1, s_d2, "z1d")

            # ---- U = raw(An @ Z1), feature-major; aug row 64 = dinv --------
            def an_prop_fm(lhs_tiles, name):
                o = act.tile([HIDDEN + 1, GPD, NPG], MMDT, tag=name)
                nc.scalar.dma_start(out=o[HIDDEN:HIDDEN + 1, :, :], in_=dinv_row)
                for h in range(0, GPD, 2):
                    p = psu.tile([HIDDEN, 2, 256], F32, tag="psu")
                    for gg in range(2):
                        g = h + gg
                        for c, cn in ((0, C0), (1, C1)):
                            off, _ = _chunk(c)
                            nc.tensor.matmul(p[:, gg, 0:NPG],
                                             lhs_tiles[c][0:cn, g, :],
                                             s_a[c][0:cn, g, :],
                                             start=(c == 0), stop=(c == 1))
                    nc.scalar.copy(o[0:HIDDEN, h:h + 2, :], p[:, :, 0:NPG])
                return o

            u = an_prop_fm(z1d, "u")

            # ---- layer 2: Z2d = d * relu((U@W2)*d + b2) --------------------
            def w_stage_nm(lhs_fm, w_aug, d2s, name, fout=HIDDEN, relu=True):
                """out[j,:] = drain(d_j^2 * sum_h lhs_aug[h,j] w_aug[h,:])."""
                outs = []
                for jc, jn in ((0, C0), (1, C1)):
                    joff, _ = _chunk(jc)
                    p = ps.tile([C0, GPD, fout], F32, tag="ps")
                    for g in range(GPD):
                        nc.tensor.matmul(p[0:jn, g, :],
                                         lhs_fm[0:HIDDEN + 1, g, joff:joff + jn],
                                         w_aug, start=True, stop=True)
                    outs.append(p)
                return outs

            p2 = w_stage_nm(u, s_W2a, s_d2, "p2")
            z2d = []
            for jc, jn in ((0, C0), (1, C1)):
                o = act.tile([jn, GPD, HIDDEN], MMDT, tag=f"z2d{jc}")
                d2bc = s_d2[jc][:][:, :, None].broadcast_to((jn, GPD, HIDDEN))
                nc.vector.scalar_tensor_tensor(
                    o[:], p2[jc][0:jn, :, :], 0.0, d2bc, AL.max, AL.mult)
                z2d.append(o)

            # ---- T = raw(An @ Z2), then P = T@Wa, softmax ------------------
            t = an_prop_fm(z2d, "t")
            pp = w_stage_nm(t, s_Waa, None, "pp", fout=CLUSTERS)

            s_S, s_Si = [], []
            for jc, jn in ((0, C0), (1, C1)):
                pm = act.tile([jn, GPD, CLUSTERS], F32, tag=f"pm{jc}")
                dbc = s_d[jc][:][:, :, None].broadcast_to((jn, GPD, CLUSTERS))
                nc.vector.tensor_mul(pm[:], pp[jc][0:jn, :, :], dbc)
                e = act.tile([jn, GPD, CLUSTERS], F32, tag=f"e{jc}")
                nc.scalar.activation(e[:], pm[:], AF.Exp)
                ssum = act.tile([jn, GPD], F32, tag=f"ssum{jc}")
                nc.vector.reduce_sum(out=ssum[:], in_=e[:],
                                     axis=mybir.AxisListType.X)
                rs = act.tile([jn, GPD], F32, tag=f"rsx{jc}")
                nc.vector.reciprocal(rs[:], ssum[:])
                s = act.tile([jn, GPD, CLUSTERS], MMDT, tag=f"s{jc}")
                nc.vector.tensor_mul(
                    s[:], e[:], rs[:][:, :, None].broadcast_to((jn, GPD, CLUSTERS)))
                s_S.append(s)
                si = act.tile([jn, GPD, CLUSTERS], MMDT, tag=f"si{jc}")
                dib = s_dinvb[jc][:][:, :, None].broadcast_to((jn, GPD, CLUSTERS))
                nc.vector.tensor_mul(si[:], s[:], dib)
                s_Si.append(si)

            # ---- AS = (A+I)@S - S, node-major ------------------------------
            s_AS = []
            for jc, jn in ((0, C0), (1, C1)):
                joff, _ = _chunk(jc)
                p = ps.tile([C0, GPD, CLUSTERS], F32, tag="ps")
                for g in range(GPD):
                    for c, cn in ((0, C0), (1, C1)):
                        nc.tensor.matmul(p[0:jn, g, :],
                                         s_a[c][0:cn, g, joff:joff + jn],
                                         s_S[c][0:cn, g, :],
                                         start=(c == 0), stop=(c == 1))
                o = act.tile([jn, GPD, CLUSTERS], MMDT, tag=f"as{jc}")
                nc.vector.tensor_sub(o[:], p[0:jn, :, :], s_S[jc][:])
                s_AS.append(o)

            # ---- Ap = S^T @ AS ; Zp^T = Z2^T @ S ---------------------------
            p_ap = ps.tile([CLUSTERS, GPD, CLUSTERS], F32, tag="ps")
            for g in range(GPD):
                for c, cn in ((0, C0), (1, C1)):
                    nc.tensor.matmul(p_ap[:, g, :], s_S[c][0:cn, g, :],
                                     s_AS[c][0:cn, g, :],
                                     start=(c == 0), stop=(c == 1))
            p_zp = ps.tile([HIDDEN, GPD, CLUSTERS], F32, tag="ps")
            for g in range(GPD):
                for c, cn in ((0, C0), (1, C1)):
                    nc.tensor.matmul(p_zp[:, g, :], z2d[c][0:cn, g, :],
                                     s_Si[c][0:cn, g, :],
                                     start=(c == 0), stop=(c == 1))
            s_Zp = act.tile([HIDDEN, GPD, CLUSTERS], MMDT, tag="zp")
            nc.scalar.copy(s_Zp[:], p_zp[:])

            # ---- pooled normalization --------------------------------------
            degp = act.tile([CLUSTERS, GPD], F32, tag="degp")
            nc.vector.reduce_sum(out=degp[:], in_=p_ap[:],
                                 axis=mybir.AxisListType.X)
            nc.vector.tensor_scalar_add(degp[:], degp[:], 1.0)
            dp = emit_rsqrt(degp, CLUSTERS, GPD, iters=1)

            # Ahp = Ap + I (node-major pooled stage: both dp factors are
            # partition scales, so no transposed dinvp row is ever needed)
            ahp = act.tile([CLUSTERS, GPD, CLUSTERS], MMDT, tag="ahp")
            id25b = idt[0:CLUSTERS, 0:CLUSTERS][:, None, :] \
                .broadcast_to((CLUSTERS, GPD, CLUSTERS))
            nc.vector.tensor_add(ahp[:], p_ap[:], id25b)

            # bp broadcast to [25, g, 64] (const, loads early)
            bp64 = cst.tile([CLUSTERS, GPD, HIDDEN], MMDT, tag="bp64")
            bpb = wpk[HIDDEN + 2:HIDDEN + 3, WP_W2A:WP_W2A + HIDDEN] \
                [:, None, :].broadcast_to((CLUSTERS, GPD, HIDDEN))
            nc.scalar.dma_start(out=bp64[:], in_=bpb)
            bc8 = cst.tile([GPD, NUM_CLASSES], F32, tag="bc8")
            nc.scalar.dma_start(
                out=bc8[:],
                in_=fpk[0:1, FP_BC:FP_BC + GPD * NUM_CLASSES]
                .rearrange("one (g c) -> (one g) c", g=GPD))

            # ---- pooled GCN: H^T = relu(dp_c' (Ahp^T ZWd) + bp), node-major
            p_zw = ps.tile([CLUSTERS, GPD, HIDDEN], F32, tag="ps")
            for g in range(GPD):
                nc.tensor.matmul(p_zw[:, g, :], s_Zp[:, g, :], s_Wp,
                                 start=True, stop=True)
            zwd = act.tile([CLUSTERS, GPD, HIDDEN], MMDT, tag="zwd")
            dpbc = dp[:][:, :, None].broadcast_to((CLUSTERS, GPD, HIDDEN))
            nc.vector.tensor_mul(zwd[:], p_zw[:], dpbc)

            p_h = ps.tile([CLUSTERS, GPD, HIDDEN], F32, tag="ps")
            for g in range(GPD):
                nc.tensor.matmul(p_h[:, g, :], ahp[:, g, :], zwd[:, g, :],
                                 start=True, stop=True)
            th = act.tile([CLUSTERS, GPD, HIDDEN], MMDT, tag="th")
            nc.vector.tensor_mul(th[:], p_h[:], dpbc)
            y2 = act.tile([CLUSTERS, GPD, HIDDEN], MMDT, tag="y2")
            nc.vector.tensor_add(y2[:], th[:], bp64[:])
            y = act.tile([CLUSTERS, GPD, HIDDEN], MMDT, tag="y")
            nc.vector.tensor_scalar_max(y[:], y2[:], 0.0)

            # ---- readout: G = sum_c' Y[c',:], logits = G @ Wc + bc ---------
            p_g = ps.tile([HIDDEN, GPD, 1], F32, tag="ps")
            for g in range(GPD):
                nc.tensor.matmul(p_g[:, g, :], y[:, g, :],
                                 warm[0:CLUSTERS, 0:1],
                                 start=True, stop=True)
            gb = act.tile([HIDDEN, GPD], MMDT, tag="gb")
            nc.scalar.copy(gb[:], p_g[:, :, 0])

            p_f = pst.tile([GPD, NUM_CLASSES], F32, tag="pf")
            nc.tensor.matmul(p_f[:], gb[:], s_Wc, start=True, stop=True)
            s_out = act.tile([GPD, NUM_CLASSES], F32, tag="logits")
            nc.vector.tensor_add(s_out[:], p_f[:], bc8[:])
            nc.sync.dma_start(out=outd, in_=s_out[:])

    nc.compile()
    return nc


def make_in_maps(x, a, W1, b1, W2, b2, Wa, ba, Wp, bp, Wc, bc):
    import ml_dtypes
    npmm = np.dtype(ml_dtypes.bfloat16)

    x = np.ascontiguousarray(np.asarray(x, dtype=np.float32))
    a = np.asarray(a, dtype=np.float32)

    ab = a.reshape(B_GRAPHS, NPG, B_GRAPHS, NPG)
    blocks = ab[np.arange(B_GRAPHS), :, np.arange(B_GRAPHS), :].copy()
    blocks[:, np.arange(NPG), np.arange(NPG)] += 1.0    # A + I
    np8 = np.dtype(ml_dtypes.float8_e4m3)
    blocks = blocks.astype(np8)

    wpk = np.zeros((N_FEAT, WP_COLS), npmm)
    wpk[:, WP_W1:WP_W1 + HIDDEN] = np.asarray(W1, np.float32).astype(npmm)
    wpk[0:HIDDEN, WP_W2A:WP_W2A + HIDDEN] = np.asarray(W2, np.float32).astype(npmm)
    wpk[HIDDEN, WP_W2A:WP_W2A + HIDDEN] = np.asarray(b2, np.float32).astype(npmm)
    wpk[0:HIDDEN, WP_WAA:WP_WAA + CLUSTERS] = np.asarray(Wa, np.float32).astype(npmm)
    wpk[HIDDEN, WP_WAA:WP_WAA + CLUSTERS] = np.asarray(ba, np.float32).astype(npmm)
    wpk[0:HIDDEN, WP_WP:WP_WP + HIDDEN] = np.asarray(Wp, np.float32).astype(npmm)
    wpk[0:HIDDEN, WP_WC:WP_WC + NUM_CLASSES] = np.asarray(Wc, np.float32).astype(npmm)
    wpk[HIDDEN + 1, WP_W2A:WP_W2A + HIDDEN] = np.asarray(b1, np.float32).astype(npmm)
    wpk[HIDDEN + 2, WP_W2A:WP_W2A + HIDDEN] = np.asarray(bp, np.float32).astype(npmm)

    fpk = np.zeros((32, FP_COLS), np.float32)
    fpk[0:CLUSTERS + 1, FP_ONES] = 1.0
    fpk[0, FP_BC:FP_BC + GPD * NUM_CLASSES] = np.tile(
        np.asarray(bc, np.float32), GPD)
    fpk[0, FP_BP64:FP_BP64 + HIDDEN] = np.asarray(bp, np.float32)

    common = dict(wpk=wpk, fpk=fpk)

    in_maps = []
    for d in range(DEV):
        xd = x[d * GPD * NPG:(d + 1) * GPD * NPG]
        xTl = np.ascontiguousarray(xd.T).reshape(N_FEAT, GPD, NPG).astype(npmm)
        bd = blocks[d * GPD:(d + 1) * GPD]
        bt = np.ascontiguousarray(bd.transpose(1, 0, 2))
        in_maps.append(dict(
            xTa=np.ascontiguousarray(xTl[:, 0:GPD // 2]),
            xTb=np.ascontiguousarray(xTl[:, GPD // 2:]),
            a0a=np.ascontiguousarray(bt[:C0, 0:GPD // 2]),
            a0b=np.ascontiguousarray(bt[:C0, GPD // 2:]),
            a1=np.ascontiguousarray(bt[C0:]),
            **common,
        ))
    return in_maps


def kernel(x, a, seg_ids, num_graphs, W1, b1, W2, b2, Wa, ba, Wp, bp, Wc, bc,
           trace=False):
    if "nc" not in _CACHE:
        _CACHE["nc"] = build_nc()
    nc = _CACHE["nc"]
    in_maps = make_in_maps(x, a, W1, b1, W2, b2, Wa, ba, Wp, bp, Wc, bc)
    res = run_bass_kernel_spmd(nc, in_maps, core_ids=list(range(DEV)), trace=trace)
    logits = np.concatenate(
        [r["out"] for r in res.results], axis=0)
    if trace:
        return logits, res
    return logits
